# revision 4
# baseline (speedup 1.0000x reference)
"""DeformUnfold (3x3, pad 1, stride 1, dil 1, DG 1) on TRN2, batch-parallel
over 8 NeuronCores.

Input  x      [8, 64, 128, 128] f32
       offset [8, 18, 128, 128] f32
Output        [8, 576, 16384]   f32  (C*K x Ho*Wo unfold, channel-major)

Per core (= one batch element):
 - Host precomputes from the offsets: wrapped int16 index lists for
   ap_gather (top-row and bottom-row corner pairs) and pair-interleaved
   bf16 weight planes (vertical lerp folded in) in gather-slot order.
 - Device: builds a bf16 pair tensor P[c, i] = (x[i], x[i+1]), duplicated
   on partitions 64..127; per (tap, chunk) gathers top/bottom pairs with
   ap_gather (d=2 bf16), then 4 DVE passes: m1 = gt*w01, m2 = gb*w23,
   s = m1+m2, out_f32 = pairsum(s); DMA result rows to the unfold layout.
 - Partition fold: Q7 groups 0-3 gather the ho<64 half of each tap while
   groups 4-7 gather ho>=64 (ap_gather cost is per index, not per
   channel), halving gather wall time.
"""

import numpy as np
import ml_dtypes

import concourse.bacc as bacc
import concourse.mybir as mybir
import concourse.tile as tile
from concourse.bass_utils import run_bass_kernel_spmd

B, C, H, W = 8, 64, 128, 128
K = 9
HW = H * W
HALF = HW // 2          # slots per half (ho<64 / ho>=64)
CH = 2048               # gather slots per chunk
NCH = HALF // CH        # chunks per (tap, half)
DT = mybir.dt

_cache = {}


def _build_nc():
    if "nc" in _cache:
        return _cache["nc"]
    nc = bacc.Bacc("TRN2", target_bir_lowering=False, debug=False)
    x_ext = nc.declare_dram_parameter("x", [C, HW], DT.float32, isOutput=False)
    idt_ext = nc.declare_dram_parameter(
        "idxt", [128, K * HALF // 16], DT.int16, isOutput=False
    )
    idb_ext = nc.declare_dram_parameter(
        "idxb", [128, K * HALF // 16], DT.int16, isOutput=False
    )
    w01_ext = nc.declare_dram_parameter(
        "w01", [2, K * HALF * 2], DT.float32, isOutput=False
    )
    w23_ext = nc.declare_dram_parameter(
        "w23", [2, K * HALF * 2], DT.float32, isOutput=False
    )
    out_ext = nc.declare_dram_parameter("out", [C * K, HW], DT.float32, isOutput=True)
    out_v = out_ext[:].rearrange("(c k) s -> c k s", k=K)

    with tile.TileContext(nc) as tc:
        with tc.tile_pool(name="img", bufs=1) as img_pool:
            P = img_pool.tile([128, HW * 2], DT.bfloat16)
            pv = P[:].rearrange("p (n d) -> p n d", d=2)
            with tc.tile_pool(name="stage", bufs=1) as stage:
                xf = stage.tile([C, HW + 1], DT.float32)
                nc.sync.dma_start(out=xf[:, 0:HW], in_=x_ext[:])
                nc.vector.memset(xf[:, HW : HW + 1], 0.0)
                for lo in (0, 64):
                    nc.vector.tensor_copy(pv[lo : lo + 64, :, 0], xf[:, 0:HW])
                    nc.vector.tensor_copy(pv[lo : lo + 64, :, 1], xf[:, 1 : HW + 1])

            with (
                tc.tile_pool(name="work", bufs=2) as work,
                tc.tile_pool(name="wpool", bufs=1) as wpool,
                tc.tile_pool(name="opool", bufs=2) as opool,
            ):
                for t in range(K):
                    for ci in range(NCH):
                        off16 = t * (HALF // 16) + ci * (CH // 16)
                        it = work.tile([128, CH // 16], DT.int16, tag="idxt")
                        nc.sync.dma_start(
                            out=it[:], in_=idt_ext[:, off16 : off16 + CH // 16]
                        )
                        ib = work.tile([128, CH // 16], DT.int16, tag="idxb")
                        nc.sync.dma_start(
                            out=ib[:], in_=idb_ext[:, off16 : off16 + CH // 16]
                        )

                        woff = (t * HALF + ci * CH) * 2
                        w01 = wpool.tile([128, CH * 2], DT.float32, tag="w01")
                        w23 = wpool.tile([128, CH * 2], DT.float32, tag="w23")
                        for wt, wext in ((w01, w01_ext), (w23, w23_ext)):
                            nc.sync.dma_start(
                                out=wt[0:64, :],
                                in_=wext[
                                    0:1, woff : woff + CH * 2
                                ].partition_broadcast(64),
                            )
                            nc.sync.dma_start(
                                out=wt[64:128, :],
                                in_=wext[
                                    1:2, woff : woff + CH * 2
                                ].partition_broadcast(64),
                            )

                        gt = work.tile([128, CH * 2], DT.bfloat16, tag="gt")
                        gb = work.tile([128, CH * 2], DT.bfloat16, tag="gb")
                        nc.gpsimd.ap_gather(
                            gt[:].rearrange("p (n d) -> p n d", d=2),
                            pv,
                            it[:],
                            channels=128,
                            num_elems=HW,
                            d=2,
                            num_idxs=CH,
                        )
                        nc.gpsimd.ap_gather(
                            gb[:].rearrange("p (n d) -> p n d", d=2),
                            pv,
                            ib[:],
                            channels=128,
                            num_elems=HW,
                            d=2,
                            num_idxs=CH,
                        )

                        nc.vector.tensor_mul(gt[:], gt[:], w01[:])
                        nc.vector.tensor_mul(gb[:], gb[:], w23[:])
                        nc.vector.tensor_add(gt[:], gt[:], gb[:])
                        sv = gt[:].rearrange("p (n d) -> p n d", d=2)
                        ot = opool.tile([128, CH], DT.float32, tag="out")
                        nc.vector.tensor_add(ot[:], sv[:, :, 0], sv[:, :, 1])

                        hbase = ci * CH
                        nc.sync.dma_start(
                            out=out_v[:, t, hbase : hbase + CH], in_=ot[0:64, :]
                        )
                        nc.sync.dma_start(
                            out=out_v[:, t, HALF + hbase : HALF + hbase + CH],
                            in_=ot[64:128, :],
                        )
    nc.compile()
    _cache["nc"] = nc
    return nc


def _host_prep(offset):
    """Per batch: wrapped idx lists (top/bottom pair rows) + premultiplied
    pair-interleaved bf16 weight planes, in gather-slot order."""
    Bn = offset.shape[0]
    ky = np.repeat(np.arange(3), 3)[None, :, None, None]
    kx = np.tile(np.arange(3), 3)[None, :, None, None]
    hs = np.arange(H)[None, None, :, None]
    ws = np.arange(W)[None, None, None, :]
    off = offset.reshape(Bn, K, 2, H, W)
    py = (ky - 1 + hs) + off[:, :, 0]
    px = (kx - 1 + ws) + off[:, :, 1]
    y0 = np.floor(py)
    x0 = np.floor(px)
    ly = (py - y0).astype(np.float32)
    lx = (px - x0).astype(np.float32)
    hy = (1.0 - ly).astype(np.float32)
    hx = (1.0 - lx).astype(np.float32)
    y0i = y0.astype(np.int64)
    x0i = x0.astype(np.int64)

    w_hy = hy * ((y0i >= 0) & (y0i < H))
    w_ly = ly * ((y0i + 1 >= 0) & (y0i + 1 < H))
    w_hx = hx * ((x0i >= 0) & (x0i < W))
    w_lx = lx * ((x0i + 1 >= 0) & (x0i + 1 < W))
    # x0 == -1: the valid x-corner (x=0) is pair slot 0 after clipping, so
    # its weight moves to slot 0 and slot 1 is dead.
    swapx = x0i == -1
    w_hx = np.where(swapx, w_lx, w_hx)
    w_lx = np.where(swapx, 0.0, w_lx)

    xc = np.clip(x0i, 0, W - 1)
    idx_top = np.clip(y0i, 0, H - 1) * W + xc
    idx_bot = np.clip(y0i + 1, 0, H - 1) * W + xc

    def wrap(a, dtype):  # [B, K, H, W] -> [B, 128, K*HALF//16]
        a = a.reshape(Bn, K, 2, HALF // 16, 16)      # s = s16*16 + p
        a = a.transpose(0, 2, 4, 1, 3)               # [B, half, p, K, s16]
        a = a.reshape(Bn, 2, 16, K * HALF // 16)
        return np.concatenate([np.repeat(a[:, 0:1], 4, 1),
                               np.repeat(a[:, 1:2], 4, 1)], axis=1).reshape(
            Bn, 128, K * HALF // 16
        ).astype(dtype)

    idx_t = wrap(idx_top, np.int16)
    idx_b = wrap(idx_bot, np.int16)

    def planes(w0, w1):  # pair-interleave -> [B, 2, K*HALF*2]
        a = np.stack([w0, w1], axis=-1)              # [B,K,H,W,2]
        a = a.reshape(Bn, K, 2, HALF, 2)
        a = a.transpose(0, 2, 1, 3, 4)               # [B, half, K, HALF, 2]
        return a.reshape(Bn, 2, K * HALF * 2).astype(np.float32)

    w01 = planes(w_hy * w_hx, w_hy * w_lx)
    w23 = planes(w_ly * w_hx, w_ly * w_lx)
    return idx_t, idx_b, w01, w23


def kernel(x, offset):
    x = np.ascontiguousarray(x, dtype=np.float32)
    offset = np.ascontiguousarray(offset, dtype=np.float32)
    idx_t, idx_b, w01, w23 = _host_prep(offset)
    nc = _build_nc()
    in_maps = [
        {
            "x": x[b].reshape(C, HW),
            "idxt": idx_t[b],
            "idxb": idx_b[b],
            "w01": w01[b],
            "w23": w23[b],
        }
        for b in range(B)
    ]
    res = run_bass_kernel_spmd(nc, in_maps, list(range(B)))
    out = np.stack([res.results[b]["out"] for b in range(B)], axis=0)
    return np.ascontiguousarray(out, dtype=np.float32)


# revision 6
# speedup vs baseline: 1.4035x; 1.4035x over previous
"""DeformUnfold (3x3, pad 1, stride 1, dil 1, DG 1) on TRN2, batch-parallel
over 8 NeuronCores.

Input  x      [8, 64, 128, 128] f32
       offset [8, 18, 128, 128] f32
Output        [8, 576, 16384]   f32  (C*K x Ho*Wo unfold, channel-major)

Per core (= one batch element):
 - Host precomputes from the offsets: wrapped int16 ap_gather index lists
   (top-row and bottom-row corner pairs, concatenated per chunk so one
   gather call serves both) and pair-interleaved bf16 weight planes with
   the vertical lerp premultiplied, in gather-slot order.
 - Device: builds a bf16 pair tensor P[c, i] = (x[i], x[i+1]) duplicated
   on partitions 64..127; per (tap, chunk) one ap_gather (d=2 bf16,
   8192 indices = 4096 top + 4096 bottom), then 3 DVE passes:
   G *= W; Gtop += Gbot; out_f32 = pairsum(Gtop); DMA to unfold layout.
 - Partition fold: Q7 groups 0-3 gather the ho<64 half of each tap while
   groups 4-7 gather ho>=64 (ap_gather cost is per index, not per
   channel), halving gather wall time.
"""

import numpy as np
import ml_dtypes

import concourse.bacc as bacc
import concourse.mybir as mybir
import concourse.tile as tile
from concourse.bass_utils import run_bass_kernel_spmd

B, C, H, W = 8, 64, 128, 128
K = 9
HW = H * W
HALF = HW // 2          # spatial slots per half (ho<64 / ho>=64)
CH = 4096               # positions per chunk (gather has 2*CH indices)
NCH = HALF // CH        # chunks per (tap, half)
DT = mybir.dt

_cache = {}


def _build_nc():
    if "nc" in _cache:
        return _cache["nc"]
    nc = bacc.Bacc("TRN2", target_bir_lowering=False, debug=False)
    x_ext = nc.declare_dram_parameter("x", [C, HW], DT.float32, isOutput=False)
    idx_ext = nc.declare_dram_parameter(
        "idx", [128, K * NCH * 2 * CH // 16], DT.int16, isOutput=False
    )
    w_ext = nc.declare_dram_parameter(
        "w", [2, K * NCH * 2 * CH * 2], DT.bfloat16, isOutput=False
    )
    out_ext = nc.declare_dram_parameter("out", [C * K, HW], DT.float32, isOutput=True)
    out_v = out_ext[:].rearrange("(c k) s -> c k s", k=K)

    with tile.TileContext(nc) as tc:
        with tc.tile_pool(name="img", bufs=1) as img_pool:
            P = img_pool.tile([128, HW * 2], DT.bfloat16)
            pv = P[:].rearrange("p (n d) -> p n d", d=2)
            with tc.tile_pool(name="stage", bufs=1) as stage:
                xf = stage.tile([C, HW + 1], DT.float32)
                nc.sync.dma_start(out=xf[:, 0:HW], in_=x_ext[:])
                nc.vector.memset(xf[:, HW : HW + 1], 0.0)
                for lo in (0, 64):
                    nc.vector.tensor_copy(pv[lo : lo + 64, :, 0], xf[:, 0:HW])
                    nc.vector.tensor_copy(pv[lo : lo + 64, :, 1], xf[:, 1 : HW + 1])

            with (
                tc.tile_pool(name="work", bufs=2) as work,
                tc.tile_pool(name="wpool", bufs=1) as wpool,
                tc.tile_pool(name="opool", bufs=2) as opool,
            ):
                for t in range(K):
                    for ci in range(NCH):
                        blk = t * NCH + ci
                        ioff = blk * (2 * CH // 16)
                        it = work.tile([128, 2 * CH // 16], DT.int16, tag="idx")
                        nc.sync.dma_start(
                            out=it[:], in_=idx_ext[:, ioff : ioff + 2 * CH // 16]
                        )

                        woff = blk * (2 * CH * 2)
                        wt = wpool.tile([128, 2 * CH * 2], DT.bfloat16, tag="w")
                        nc.sync.dma_start(
                            out=wt[0:64, :],
                            in_=w_ext[0:1, woff : woff + 2 * CH * 2].partition_broadcast(64),
                        )
                        nc.sync.dma_start(
                            out=wt[64:128, :],
                            in_=w_ext[1:2, woff : woff + 2 * CH * 2].partition_broadcast(64),
                        )

                        g = work.tile([128, 2 * CH * 2], DT.bfloat16, tag="g")
                        nc.gpsimd.ap_gather(
                            g[:].rearrange("p (n d) -> p n d", d=2),
                            pv,
                            it[:],
                            channels=128,
                            num_elems=HW,
                            d=2,
                            num_idxs=2 * CH,
                        )

                        nc.vector.tensor_mul(g[:], g[:], wt[:])
                        gtop = g[:, 0 : CH * 2]
                        gbot = g[:, CH * 2 : 2 * CH * 2]
                        nc.vector.tensor_add(gtop, gtop, gbot)
                        sv = gtop.rearrange("p (n d) -> p n d", d=2)
                        ot = opool.tile([128, CH], DT.float32, tag="out")
                        nc.vector.tensor_add(ot[:], sv[:, :, 0], sv[:, :, 1])

                        hbase = ci * CH
                        nc.sync.dma_start(
                            out=out_v[:, t, hbase : hbase + CH], in_=ot[0:64, :]
                        )
                        nc.sync.dma_start(
                            out=out_v[:, t, HALF + hbase : HALF + hbase + CH],
                            in_=ot[64:128, :],
                        )
    nc.compile()
    _cache["nc"] = nc
    return nc


def _host_prep(offset):
    """Per batch: wrapped idx lists (top|bottom concatenated per chunk) +
    premultiplied pair-interleaved bf16 weights in gather-slot order."""
    Bn = offset.shape[0]
    ky = np.repeat(np.arange(3), 3)[None, :, None, None]
    kx = np.tile(np.arange(3), 3)[None, :, None, None]
    hs = np.arange(H)[None, None, :, None]
    ws = np.arange(W)[None, None, None, :]
    off = offset.reshape(Bn, K, 2, H, W)
    py = (ky - 1 + hs) + off[:, :, 0]
    px = (kx - 1 + ws) + off[:, :, 1]
    y0 = np.floor(py)
    x0 = np.floor(px)
    ly = (py - y0).astype(np.float32)
    lx = (px - x0).astype(np.float32)
    hy = (1.0 - ly).astype(np.float32)
    hx = (1.0 - lx).astype(np.float32)
    y0i = y0.astype(np.int64)
    x0i = x0.astype(np.int64)

    w_hy = hy * ((y0i >= 0) & (y0i < H))
    w_ly = ly * ((y0i + 1 >= 0) & (y0i + 1 < H))
    w_hx = hx * ((x0i >= 0) & (x0i < W))
    w_lx = lx * ((x0i + 1 >= 0) & (x0i + 1 < W))
    # x0 == -1: after clipping, the valid x-corner (x=0) sits in pair slot 0,
    # so its weight moves to slot 0 and slot 1 is dead.
    swapx = x0i == -1
    w_hx = np.where(swapx, w_lx, w_hx)
    w_lx = np.where(swapx, 0.0, w_lx)

    xc = np.clip(x0i, 0, W - 1)
    idx_top = np.clip(y0i, 0, H - 1) * W + xc
    idx_bot = np.clip(y0i + 1, 0, H - 1) * W + xc

    def wrap(a):  # [B, K, H, W] -> [B, half(2), 16, K, NCH, CH//16]
        a = a.reshape(Bn, K, 2, NCH, CH // 16, 16)   # s = (ci, s16, p)
        return a.transpose(0, 2, 5, 1, 3, 4)

    wt_ = wrap(idx_top)
    wb_ = wrap(idx_bot)
    # concat top|bottom per (K, NCH) chunk -> free = (K, NCH, 2, CH//16)
    cat = np.stack([wt_, wb_], axis=5)               # [B,2,16,K,NCH,2,CH//16]
    cat = cat.reshape(Bn, 2, 16, K * NCH * 2 * CH // 16)
    idx_w = np.concatenate(
        [np.repeat(cat[:, 0:1], 4, 1), np.repeat(cat[:, 1:2], 4, 1)], axis=1
    ).reshape(Bn, 128, K * NCH * 2 * CH // 16).astype(np.int16)

    def plane(w0, w1):  # [B, K, H, W] x2 -> [B, half, K, NCH, CH, 2]
        a = np.stack([w0, w1], axis=-1)
        a = a.reshape(Bn, K, 2, NCH, CH, 2)
        return a.transpose(0, 2, 1, 3, 4, 5)

    ptop = plane(w_hy * w_hx, w_hy * w_lx)
    pbot = plane(w_ly * w_hx, w_ly * w_lx)
    wcat = np.stack([ptop, pbot], axis=4)            # [B,2,K,NCH,2,CH,2]
    w_pl = wcat.reshape(Bn, 2, K * NCH * 2 * CH * 2).astype(ml_dtypes.bfloat16)
    return idx_w, w_pl


def kernel(x, offset):
    x = np.ascontiguousarray(x, dtype=np.float32)
    offset = np.ascontiguousarray(offset, dtype=np.float32)
    idx_w, w_pl = _host_prep(offset)
    nc = _build_nc()
    in_maps = [
        {"x": x[b].reshape(C, HW), "idx": idx_w[b], "w": w_pl[b]}
        for b in range(B)
    ]
    res = run_bass_kernel_spmd(nc, in_maps, list(range(B)))
    out = np.stack([res.results[b]["out"] for b in range(B)], axis=0)
    return np.ascontiguousarray(out, dtype=np.float32)


# revision 7
# speedup vs baseline: 3.4124x; 2.4314x over previous
"""DeformUnfold (3x3, pad 1, stride 1, dil 1, DG 1) on TRN2, batch-parallel
over 8 NeuronCores.

Input  x      [8, 64, 128, 128] f32
       offset [8, 18, 128, 128] f32
Output        [8, 576, 16384]   f32  (C*K x Ho*Wo unfold, channel-major)

Per core (= one batch element):
 - Host precomputes from the offsets: wrapped int16 ap_gather index lists
   (top-row and bottom-row corner pairs, concatenated per chunk so one
   gather call serves both) and pair-interleaved fp16 weight planes with
   the vertical lerp premultiplied, in gather-slot order.
 - Device: builds a fp16 pair tensor P[c, i] = (x[i], x[i+1]) duplicated
   on partitions 64..127; per (tap, chunk) one ap_gather (d=2 fp16,
   8192 indices = 4096 top + 4096 bottom), then 3 DVE passes:
   G *= W; Gtop += Gbot; out_f32 = pairsum(Gtop); DMA to unfold layout.
 - Partition fold: Q7 groups 0-3 gather the ho<64 half of each tap while
   groups 4-7 gather ho>=64 (ap_gather cost is per index, not per
   channel), halving gather wall time.
"""

import numpy as np
import ml_dtypes

import concourse.bacc as bacc
import concourse.mybir as mybir
import concourse.tile as tile
from concourse.bass_utils import run_bass_kernel_spmd

B, C, H, W = 8, 64, 128, 128
K = 9
HW = H * W
HALF = HW // 2          # spatial slots per half (ho<64 / ho>=64)
CH = 4096               # positions per chunk (gather has 2*CH indices)
NCH = HALF // CH        # chunks per (tap, half)
DT = mybir.dt

_cache = {}


def _build_nc():
    if "nc" in _cache:
        return _cache["nc"]
    nc = bacc.Bacc("TRN2", target_bir_lowering=False, debug=False)
    x_ext = nc.declare_dram_parameter("x", [C, HW], DT.float32, isOutput=False)
    idx_ext = nc.declare_dram_parameter(
        "idx", [128, K * NCH * 2 * CH // 16], DT.int16, isOutput=False
    )
    w_ext = nc.declare_dram_parameter(
        "w", [2, K * NCH * 2 * CH * 2], DT.float16, isOutput=False
    )
    out_ext = nc.declare_dram_parameter("out", [C * K, HW], DT.float32, isOutput=True)
    out_v = out_ext[:].rearrange("(c k) s -> c k s", k=K)

    with tile.TileContext(nc) as tc:
        with tc.tile_pool(name="img", bufs=1) as img_pool:
            P = img_pool.tile([128, HW * 2], DT.float16)
            pv = P[:].rearrange("p (n d) -> p n d", d=2)
            with tc.tile_pool(name="stage", bufs=1) as stage:
                xf = stage.tile([C, HW + 1], DT.float32)
                nc.sync.dma_start(out=xf[:, 0:HW], in_=x_ext[:])
                nc.vector.memset(xf[:, HW : HW + 1], 0.0)
                for lo in (0, 64):
                    nc.vector.tensor_copy(pv[lo : lo + 64, :, 0], xf[:, 0:HW])
                    nc.vector.tensor_copy(pv[lo : lo + 64, :, 1], xf[:, 1 : HW + 1])

            with (
                tc.tile_pool(name="work", bufs=2) as work,
                tc.tile_pool(name="wpool", bufs=1) as wpool,
                tc.tile_pool(name="opool", bufs=2) as opool,
            ):
                for t in range(K):
                    for ci in range(NCH):
                        blk = t * NCH + ci
                        ioff = blk * (2 * CH // 16)
                        it = work.tile([128, 2 * CH // 16], DT.int16, tag="idx")
                        nc.sync.dma_start(
                            out=it[:], in_=idx_ext[:, ioff : ioff + 2 * CH // 16]
                        )

                        woff = blk * (2 * CH * 2)
                        wt = wpool.tile([128, 2 * CH * 2], DT.float16, tag="w")
                        nc.sync.dma_start(
                            out=wt[0:64, :],
                            in_=w_ext[0:1, woff : woff + 2 * CH * 2].partition_broadcast(64),
                        )
                        nc.sync.dma_start(
                            out=wt[64:128, :],
                            in_=w_ext[1:2, woff : woff + 2 * CH * 2].partition_broadcast(64),
                        )

                        g = work.tile([128, 2 * CH * 2], DT.float16, tag="g")
                        nc.gpsimd.ap_gather(
                            g[:].rearrange("p (n d) -> p n d", d=2),
                            pv,
                            it[:],
                            channels=128,
                            num_elems=HW,
                            d=2,
                            num_idxs=2 * CH,
                        )

                        nc.vector.tensor_mul(g[:], g[:], wt[:])
                        gtop = g[:, 0 : CH * 2]
                        gbot = g[:, CH * 2 : 2 * CH * 2]
                        nc.vector.tensor_add(gtop, gtop, gbot)
                        sv = gtop.rearrange("p (n d) -> p n d", d=2)
                        ot = opool.tile([128, CH], DT.float32, tag="out")
                        nc.vector.tensor_add(ot[:], sv[:, :, 0], sv[:, :, 1])

                        hbase = ci * CH
                        nc.sync.dma_start(
                            out=out_v[:, t, hbase : hbase + CH], in_=ot[0:64, :]
                        )
                        nc.sync.dma_start(
                            out=out_v[:, t, HALF + hbase : HALF + hbase + CH],
                            in_=ot[64:128, :],
                        )
    nc.compile()
    _cache["nc"] = nc
    return nc


def _host_prep(offset):
    """Per batch: wrapped idx lists (top|bottom concatenated per chunk) +
    premultiplied pair-interleaved fp16 weights in gather-slot order."""
    Bn = offset.shape[0]
    ky = np.repeat(np.arange(3), 3)[None, :, None, None]
    kx = np.tile(np.arange(3), 3)[None, :, None, None]
    hs = np.arange(H)[None, None, :, None]
    ws = np.arange(W)[None, None, None, :]
    off = offset.reshape(Bn, K, 2, H, W)
    py = (ky - 1 + hs) + off[:, :, 0]
    px = (kx - 1 + ws) + off[:, :, 1]
    y0 = np.floor(py)
    x0 = np.floor(px)
    ly = (py - y0).astype(np.float32)
    lx = (px - x0).astype(np.float32)
    hy = (1.0 - ly).astype(np.float32)
    hx = (1.0 - lx).astype(np.float32)
    y0i = y0.astype(np.int64)
    x0i = x0.astype(np.int64)

    w_hy = hy * ((y0i >= 0) & (y0i < H))
    w_ly = ly * ((y0i + 1 >= 0) & (y0i + 1 < H))
    w_hx = hx * ((x0i >= 0) & (x0i < W))
    w_lx = lx * ((x0i + 1 >= 0) & (x0i + 1 < W))
    # x0 == -1: after clipping, the valid x-corner (x=0) sits in pair slot 0,
    # so its weight moves to slot 0 and slot 1 is dead.
    swapx = x0i == -1
    w_hx = np.where(swapx, w_lx, w_hx)
    w_lx = np.where(swapx, 0.0, w_lx)

    xc = np.clip(x0i, 0, W - 1)
    idx_top = np.clip(y0i, 0, H - 1) * W + xc
    idx_bot = np.clip(y0i + 1, 0, H - 1) * W + xc

    def wrap(a):  # [B, K, H, W] -> [B, half(2), 16, K, NCH, CH//16]
        a = a.reshape(Bn, K, 2, NCH, CH // 16, 16)   # s = (ci, s16, p)
        return a.transpose(0, 2, 5, 1, 3, 4)

    wt_ = wrap(idx_top)
    wb_ = wrap(idx_bot)
    # concat top|bottom per (K, NCH) chunk -> free = (K, NCH, 2, CH//16)
    cat = np.stack([wt_, wb_], axis=5)               # [B,2,16,K,NCH,2,CH//16]
    cat = cat.reshape(Bn, 2, 16, K * NCH * 2 * CH // 16)
    idx_w = np.concatenate(
        [np.repeat(cat[:, 0:1], 4, 1), np.repeat(cat[:, 1:2], 4, 1)], axis=1
    ).reshape(Bn, 128, K * NCH * 2 * CH // 16).astype(np.int16)

    def plane(w0, w1):  # [B, K, H, W] x2 -> [B, half, K, NCH, CH, 2]
        a = np.stack([w0, w1], axis=-1)
        a = a.reshape(Bn, K, 2, NCH, CH, 2)
        return a.transpose(0, 2, 1, 3, 4, 5)

    ptop = plane(w_hy * w_hx, w_hy * w_lx)
    pbot = plane(w_ly * w_hx, w_ly * w_lx)
    wcat = np.stack([ptop, pbot], axis=4)            # [B,2,K,NCH,2,CH,2]
    w_pl = wcat.reshape(Bn, 2, K * NCH * 2 * CH * 2).astype(np.float16)
    return idx_w, w_pl


def kernel(x, offset):
    x = np.ascontiguousarray(x, dtype=np.float32)
    offset = np.ascontiguousarray(offset, dtype=np.float32)
    idx_w, w_pl = _host_prep(offset)
    nc = _build_nc()
    in_maps = [
        {"x": x[b].reshape(C, HW), "idx": idx_w[b], "w": w_pl[b]}
        for b in range(B)
    ]
    res = run_bass_kernel_spmd(nc, in_maps, list(range(B)))
    out = np.stack([res.results[b]["out"] for b in range(B)], axis=0)
    return np.ascontiguousarray(out, dtype=np.float32)
